# revision 9
# baseline (speedup 1.0000x reference)
"""BiLSTM-CRF Trainium2 kernel.

Strategy:
- 8 cores, one SPMD launch for the BiLSTM: 4 forward-sequence chunks on cores
  0-3, 4 reversed-sequence chunks on cores 4-7 (backward LSTM = forward LSTM on
  the host-reversed sentence). Each core processes CHUNK=1024 positions plus a
  HALO=128 left halo; the LSTM is contractive (per-step Jacobian norm ~0.55),
  so the halo decays entry-state error to ~1e-17 and chunks are independent —
  no collectives.
- Within a core the sequential LSTM recurrence is solved by Jacobi fixed-point
  iteration: gates_t = xg_t + W_hh @ h_{t-1} evaluated for all t at once as a
  dense matmul (t is the moving free dim), K_ITERS times. The linear c-
  recurrence is solved exactly each iteration with the hardware prefix-scan
  instruction (tensor_tensor_scan). ~16 iterations converge to fp32 roundoff.
- All matmuls use bf16 hi/lo split operands (3 cross products, fp32
  accumulate) for ~fp32-accurate products.
- Second tiny launch (1 core): blocked Viterbi. 128 partitions x 32 steps:
  per-block max-plus transfer matrices, lumped exclusive scan for block entry
  vectors, then 2 refinement passes that recompute fv within blocks (entries
  fed from the previous pass's block exits), extracting argmax backpointers
  with the reference's first-index tie-breaking. Backtrack is parallelized by
  composing the per-step 7->7 backpointer maps (one-hot select trick), and the
  score is recomputed as the path sum (tree reduction).
"""

import numpy as np
import ml_dtypes
from contextlib import ExitStack

import concourse.bass as bass
import concourse.mybir as mybir
import concourse.tile as tile
from concourse import bacc
from concourse.masks import make_identity

F32 = mybir.dt.float32
BF16 = mybir.dt.bfloat16
I32 = mybir.dt.int32
AF = mybir.ActivationFunctionType
OP = mybir.AluOpType

VOCAB, E, HID, H, T, L = 100000, 512, 1024, 512, 7, 4096
START, STOP, NEG = 4, 5, -10000.0
NEG2 = -1.0e6
P = 128
NCORES = 8

CHUNK = 1024
HALO = 128
LC = CHUNK + HALO          # 1152 positions per core
K_ITERS = 16
KCH = H // P               # 4 h-chunks (k stripes)
MT = 4 * H // P            # 16 gate tiles
NB, S = 128, 32            # viterbi blocks / steps per block
ROUNDS = 2

LAST_RESULTS = []          # exec_time_ns per launch, for test harness


def _chunks(lc):
    out, n0 = [], 0
    while n0 < lc:
        nl = min(512, lc - n0)
        out.append((n0, nl))
        n0 += nl
    return out


# --------------------------------------------------------------------------
# Launch 1: per-chunk LSTM (SPMD x8)
# --------------------------------------------------------------------------
def build_lstm(lc=LC, halo=HALO, k_iters=K_ITERS):
    chunk = lc - halo
    ng = lc // P
    assert lc % P == 0 and chunk % P == 0
    nc = bacc.Bacc("TRN2", target_bir_lowering=False, debug=False)

    emb = nc.dram_tensor("emb", [VOCAB, E], F32, kind="ExternalInput")
    idx = nc.dram_tensor("idx", [lc, 1], I32, kind="ExternalInput")
    mask128 = nc.dram_tensor("mask128", [P, halo], F32, kind="ExternalInput")
    wih_hi = nc.dram_tensor("wih_hi", [E, 4 * H], BF16, kind="ExternalInput")
    wih_lo = nc.dram_tensor("wih_lo", [E, 4 * H], BF16, kind="ExternalInput")
    whh_hi = nc.dram_tensor("whh_hi", [H, 4 * H], BF16, kind="ExternalInput")
    whh_lo = nc.dram_tensor("whh_lo", [H, 4 * H], BF16, kind="ExternalInput")
    bias = nc.dram_tensor("bias", [P, MT], F32, kind="ExternalInput")
    wo_hi = nc.dram_tensor("wo_hi", [H, T], BF16, kind="ExternalInput")
    wo_lo = nc.dram_tensor("wo_lo", [H, T], BF16, kind="ExternalInput")
    feats_out = nc.dram_tensor("feats_out", [chunk, T], F32, kind="ExternalOutput")

    chs = _chunks(lc)

    with tile.TileContext(nc) as tc, ExitStack() as ctx:
        stA = ctx.enter_context(tc.tile_pool(name="stA", bufs=1))
        dr = ctx.enter_context(tc.tile_pool(name="dr", bufs=1, space="DRAM"))
        psA = ctx.enter_context(tc.tile_pool(name="psA", bufs=2, space="PSUM"))
        psB = ctx.enter_context(tc.tile_pool(name="psB", bufs=2, space="PSUM"))

        whh_sb = {}
        for nm, dt_ in (("hi", whh_hi), ("lo", whh_lo)):
            t_ = stA.tile([P, KCH * 4 * H], BF16, tag=f"whh{nm}", name=f"whhs{nm}")
            for k in range(KCH):
                nc.sync.dma_start(t_[:, k * 4 * H:(k + 1) * 4 * H],
                                  dt_[k * P:(k + 1) * P, :])
            whh_sb[nm] = t_
        wo_sb = {}
        for nm, dt_ in (("hi", wo_hi), ("lo", wo_lo)):
            t_ = stA.tile([P, KCH * T], BF16, tag=f"wo{nm}", name=f"wos{nm}")
            for k in range(KCH):
                nc.sync.dma_start(t_[:, k * T:(k + 1) * T], dt_[k * P:(k + 1) * P, :])
            wo_sb[nm] = t_

        xg_dram = dr.tile([4 * H, lc], F32)

        # ---- phase 1: gather + transpose + xg build (pool freed after) ----
        with tc.tile_pool(name="xw", bufs=1) as xw, \
             tc.tile_pool(name="gst", bufs=3) as gst_p, \
             tc.tile_pool(name="xgst", bufs=3) as xgst_p:
            ident = xw.tile([P, P], F32)
            make_identity(nc, ident[:])
            bias_sb = xw.tile([P, MT], F32)
            nc.sync.dma_start(bias_sb[:], bias[:])
            mask_sb = xw.tile([P, halo], F32)
            nc.sync.dma_start(mask_sb[:], mask128[:])
            idx_sb = xw.tile([P, ng], I32)
            nc.sync.dma_start(idx_sb[:].unsqueeze(2),
                              idx[:].rearrange("(g p) o -> p g o", p=P))
            wih_sb = {}
            for nm, dt_ in (("hi", wih_hi), ("lo", wih_lo)):
                t_ = xw.tile([P, KCH * 4 * H], BF16, tag=f"wih{nm}", name=f"wihs{nm}")
                for k in range(KCH):
                    nc.sync.dma_start(t_[:, k * 4 * H:(k + 1) * 4 * H],
                                      dt_[k * P:(k + 1) * P, :])
                wih_sb[nm] = t_
            xT = {"hi": xw.tile([P, KCH * lc], BF16, tag="xTh", name="xTh"),
                  "lo": xw.tile([P, KCH * lc], BF16, tag="xTl", name="xTl")}
            for g in range(ng):
                gt = gst_p.tile([P, E], F32)
                nc.gpsimd.indirect_dma_start(
                    out=gt[:], out_offset=None, in_=emb[:],
                    in_offset=bass.IndirectOffsetOnAxis(ap=idx_sb[:, g:g + 1], axis=0))
                for k in range(KCH):
                    pt = psB.tile([P, P], F32, space="PSUM")
                    nc.tensor.transpose(out=pt[:], in_=gt[:, k * P:(k + 1) * P],
                                        identity=ident[:])
                    hi_sl = xT["hi"][:, k * lc + g * P: k * lc + (g + 1) * P]
                    nc.vector.tensor_copy(out=hi_sl, in_=pt[:])
                    nc.vector.tensor_tensor(
                        out=xT["lo"][:, k * lc + g * P: k * lc + (g + 1) * P],
                        in0=pt[:], in1=hi_sl, op=OP.subtract)
            for m in range(MT):
                for (n0, nl) in chs:
                    ps = psA.tile([P, 512], F32, space="PSUM")
                    first = True
                    for k in range(KCH):
                        for a, b in (("hi", "hi"), ("hi", "lo"), ("lo", "hi")):
                            nc.tensor.matmul(
                                out=ps[:, :nl],
                                lhsT=wih_sb[a][:, k * 4 * H + m * P: k * 4 * H + (m + 1) * P],
                                rhs=xT[b][:, k * lc + n0: k * lc + n0 + nl],
                                start=first, stop=(k == KCH - 1 and (a, b) == ("lo", "hi")))
                            first = False
                    stage = xgst_p.tile([P, 512], F32, tag="xgst")
                    nc.scalar.activation(stage[:, :nl], ps[:, :nl], AF.Identity,
                                         bias=bias_sb[:, m:m + 1], scale=1.0)
                    if n0 == 0:
                        nc.vector.tensor_tensor(out=stage[:, :halo], in0=stage[:, :halo],
                                                in1=mask_sb[:], op=OP.mult)
                    nc.sync.dma_start(xg_dram[m * P:(m + 1) * P, n0:n0 + nl],
                                      stage[:, :nl])

        # ---- phase 2: Jacobi iterations ----
        stB = ctx.enter_context(tc.tile_pool(name="stB", bufs=1))
        g1p = ctx.enter_context(tc.tile_pool(name="g1p", bufs=3))
        tip = ctx.enter_context(tc.tile_pool(name="tip", bufs=2))
        tgp = ctx.enter_context(tc.tile_pool(name="tgp", bufs=2))
        xgin = ctx.enter_context(tc.tile_pool(name="xgin", bufs=3))
        fsp = ctx.enter_context(tc.tile_pool(name="fsp", bufs=2))

        h_sb = {"hi": stB.tile([P, KCH * (lc + 1)], BF16, tag="hhi", name="hhi"),
                "lo": stB.tile([P, KCH * (lc + 1)], BF16, tag="hlo", name="hlo")}
        nc.gpsimd.memset(h_sb["hi"][:], 0.0)
        nc.gpsimd.memset(h_sb["lo"][:], 0.0)
        fbuf = stB.tile([P, KCH * lc], F32, tag="fbuf")
        ubuf = stB.tile([P, KCH * lc], F32, tag="ubuf")
        obuf = stB.tile([P, KCH * lc], F32, tag="obuf")

        for it in range(k_iters):
            for (n0, nl) in chs:
                for j in range(KCH):
                    acts = {}
                    for gate, m in (("i", j), ("g", 2 * KCH + j),
                                    ("f", KCH + j), ("o", 3 * KCH + j)):
                        ps = psA.tile([P, 512], F32, space="PSUM")
                        first = True
                        for k in range(KCH):
                            for a, b in (("hi", "hi"), ("hi", "lo"), ("lo", "hi")):
                                nc.tensor.matmul(
                                    out=ps[:, :nl],
                                    lhsT=whh_sb[a][:, k * 4 * H + m * P: k * 4 * H + (m + 1) * P],
                                    rhs=h_sb[b][:, k * (lc + 1) + n0: k * (lc + 1) + n0 + nl],
                                    start=first,
                                    stop=(k == KCH - 1 and (a, b) == ("lo", "hi")))
                                first = False
                        xg_t = xgin.tile([P, 512], F32, tag="xgin")
                        nc.sync.dma_start(xg_t[:, :nl],
                                          xg_dram[m * P:(m + 1) * P, n0:n0 + nl])
                        g1 = g1p.tile([P, 512], F32, tag="g1")
                        nc.vector.tensor_tensor(out=g1[:, :nl], in0=ps[:, :nl],
                                                in1=xg_t[:, :nl], op=OP.add)
                        sl = slice(j * lc + n0, j * lc + n0 + nl)
                        if gate == "i":
                            ti = tip.tile([P, 512], F32, tag="ti")
                            nc.scalar.activation(ti[:, :nl], g1[:, :nl], AF.Sigmoid)
                            acts["i"] = ti
                        elif gate == "g":
                            tg = tgp.tile([P, 512], F32, tag="tg")
                            nc.scalar.activation(tg[:, :nl], g1[:, :nl], AF.Tanh)
                            nc.vector.tensor_tensor(out=ubuf[:, sl],
                                                    in0=acts["i"][:, :nl],
                                                    in1=tg[:, :nl], op=OP.mult)
                        elif gate == "f":
                            nc.scalar.activation(fbuf[:, sl], g1[:, :nl], AF.Sigmoid)
                        else:
                            nc.scalar.activation(obuf[:, sl], g1[:, :nl], AF.Sigmoid)
            for j in range(KCH):
                sl = slice(j * lc, (j + 1) * lc)
                # c-scan in place over u (data1): c_t = f_t*c_{t-1} + u_t
                nc.vector.tensor_tensor_scan(out=ubuf[:, sl], data0=fbuf[:, sl],
                                             data1=ubuf[:, sl], initial=0.0,
                                             op0=OP.mult, op1=OP.add)
                nc.scalar.activation(fbuf[:, sl], ubuf[:, sl], AF.Tanh)
                nc.vector.tensor_tensor(out=obuf[:, sl], in0=obuf[:, sl],
                                        in1=fbuf[:, sl], op=OP.mult)
                hsl = slice(j * (lc + 1) + 1, j * (lc + 1) + 1 + lc)
                nc.vector.tensor_copy(out=h_sb["hi"][:, hsl], in_=obuf[:, sl])
                nc.vector.tensor_tensor(out=h_sb["lo"][:, hsl], in0=obuf[:, sl],
                                        in1=h_sb["hi"][:, hsl], op=OP.subtract)

        # ---- feats projection (skip halo) ----
        for mt_i in range(chunk // P):
            ps = psB.tile([P, P], F32, space="PSUM")
            first = True
            for k in range(KCH):
                for a, b in (("hi", "hi"), ("hi", "lo"), ("lo", "hi")):
                    nc.tensor.matmul(
                        out=ps[:, :T],
                        lhsT=h_sb[a][:, k * (lc + 1) + 1 + halo + mt_i * P:
                                     k * (lc + 1) + 1 + halo + (mt_i + 1) * P],
                        rhs=wo_sb[b][:, k * T:(k + 1) * T],
                        start=first, stop=(k == KCH - 1 and (a, b) == ("lo", "hi")))
                    first = False
            fs = fsp.tile([P, T], F32, tag="fs")
            nc.scalar.activation(fs[:], ps[:, :T], AF.Copy)
            nc.sync.dma_start(feats_out[mt_i * P:(mt_i + 1) * P, :], fs[:])

    nc.compile()
    return nc


# --------------------------------------------------------------------------
# Launch 2: blocked Viterbi decode (1 core)
# --------------------------------------------------------------------------
def build_viterbi(rounds=ROUNDS):
    nc = bacc.Bacc("TRN2", target_bir_lowering=False, debug=False)

    fA = nc.dram_tensor("fA", [L, T], F32, kind="ExternalInput")
    fB = nc.dram_tensor("fB", [L, T], F32, kind="ExternalInput")
    b_out_t = nc.dram_tensor("b_out_t", [P, S * T], F32, kind="ExternalInput")
    trans49 = nc.dram_tensor("trans49", [P, 49], F32, kind="ExternalInput")
    iota_p49 = nc.dram_tensor("iota_p49", [P, 49], F32, kind="ExternalInput")
    iota49l = nc.dram_tensor("iota49l", [P, 49], F32, kind="ExternalInput")
    id49 = nc.dram_tensor("id49", [P, 49], F32, kind="ExternalInput")
    iota7 = nc.dram_tensor("iota7", [P, T], F32, kind="ExternalInput")
    init7 = nc.dram_tensor("init7", [P, T], F32, kind="ExternalInput")
    trSTOP = nc.dram_tensor("trSTOP", [P, T], F32, kind="ExternalInput")
    path_out = nc.dram_tensor("path_out", [P, S], I32, kind="ExternalOutput")
    score_out = nc.dram_tensor("score_out", [1, 1], F32, kind="ExternalOutput")

    def r3(ap, names="p (a b) -> p a b", a=7):
        return ap.rearrange(names, a=a)

    with tile.TileContext(nc) as tc, ExitStack() as ctx:
        st = ctx.enter_context(tc.tile_pool(name="st", bufs=1))
        wk = ctx.enter_context(tc.tile_pool(name="wk", bufs=4))
        pp = ctx.enter_context(tc.tile_pool(name="pp", bufs=4))
        ps_p = ctx.enter_context(tc.tile_pool(name="psv", bufs=2, space="PSUM"))

        # constants
        cons = {}
        for nm, dt_, w in (("trans49", trans49, 49), ("iota_p49", iota_p49, 49),
                           ("iota49l", iota49l, 49), ("id49", id49, 49),
                           ("iota7", iota7, T), ("init7", init7, T),
                           ("trSTOP", trSTOP, T), ("b_out_t", b_out_t, S * T)):
            t_ = st.tile([P, w], F32, tag=nm)
            nc.sync.dma_start(t_[:], dt_[:])
            cons[nm] = t_
        trans_v = r3(cons["trans49"][:])        # [p, n, k]
        iota_p_v = r3(cons["iota_p49"][:])      # [p, x, y] value y

        # feats [128, 224]
        fsb = st.tile([P, S * T], F32, tag="fsb")
        ta = wk.tile([P, S * T], F32, tag="fa")
        tb = wk.tile([P, S * T], F32, tag="fb")
        nc.sync.dma_start(ta[:].rearrange("p (s n) -> p s n", s=S),
                          fA[:].rearrange("(b s) n -> b s n", s=S))
        nc.sync.dma_start(tb[:].rearrange("p (s n) -> p s n", s=S),
                          fB[:].rearrange("(b s) n -> b s n", s=S))
        nc.vector.tensor_tensor(out=fsb[:], in0=ta[:], in1=tb[:], op=OP.add)
        nc.vector.tensor_tensor(out=fsb[:], in0=fsb[:], in1=cons["b_out_t"][:], op=OP.add)

        def mx_expand_iNk(Rv):
            # T1[p,i,n,k] = R[p,i,k] + trans[p,n,k]; returns [P,343] tile
            t1 = pp.tile([P, 343], F32, tag="t343")
            nc.vector.tensor_tensor(
                out=t1[:].rearrange("p (i n k) -> p i n k", i=7, n=7),
                in0=r3(Rv).unsqueeze(2).broadcast_to([P, 7, 7, 7]),
                in1=trans_v.unsqueeze(1).broadcast_to([P, 7, 7, 7]),
                op=OP.add)
            return t1

        # ---- pass 1: per-block transfer matrices ----
        R = st.tile([P, 49], F32, tag="R")
        nc.vector.tensor_copy(out=R[:], in_=cons["id49"][:])
        for s in range(S):
            t1 = mx_expand_iNk(R[:])
            m1 = pp.tile([P, 49], F32, tag="m49")
            nc.vector.tensor_reduce(out=r3(m1[:]),
                                    in_=t1[:].rearrange("p (i n k) -> p i n k", i=7, n=7),
                                    axis=mybir.AxisListType.X, op=OP.max)
            nc.vector.tensor_tensor(
                out=r3(R[:]),
                in0=r3(m1[:]),
                in1=fsb[:, s * T:(s + 1) * T].unsqueeze(1).broadcast_to([P, 7, 7]),
                op=OP.add)

        # ---- pass 2: lumped exclusive max-plus scan across partitions ----
        Pc = st.tile([P, 49], F32, tag="Pc")
        nc.vector.tensor_copy(out=Pc[:], in_=R[:])
        for d in (1, 2, 4, 8, 16, 32, 64):
            Psh = pp.tile([P, 49], F32, tag="Psh")
            nc.sync.dma_start(Psh[d:P, :], Pc[0:P - d, :])
            nc.sync.dma_start(Psh[0:d, :], cons["id49"][0:d, :])
            t1 = pp.tile([P, 343], F32, tag="t343")
            nc.vector.tensor_tensor(
                out=t1[:].rearrange("p (i j k) -> p i j k", i=7, j=7),
                in0=r3(Psh[:]).unsqueeze(2).broadcast_to([P, 7, 7, 7]),
                in1=r3(Pc[:]).transpose([0, 2, 1])
                    .unsqueeze(1).broadcast_to([P, 7, 7, 7]),
                op=OP.add)
            Pn = pp.tile([P, 49], F32, tag="Pn")
            nc.vector.tensor_reduce(out=r3(Pn[:]),
                                    in_=t1[:].rearrange("p (i j k) -> p i j k", i=7, j=7),
                                    axis=mybir.AxisListType.X, op=OP.max)
            nc.vector.tensor_copy(out=Pc[:], in_=Pn[:])
        Pex = st.tile([P, 49], F32, tag="Pex")
        nc.sync.dma_start(Pex[1:P, :], Pc[0:P - 1, :])
        nc.sync.dma_start(Pex[0:1, :], cons["id49"][0:1, :])
        # entries e[k] = max_i(init[i] + Pex[i,k])
        ent = st.tile([P, T], F32, tag="ent")
        t2 = pp.tile([P, 49], F32, tag="t49b")
        nc.vector.tensor_tensor(
            out=r3(t2[:]),
            in0=cons["init7"][:].unsqueeze(1).broadcast_to([P, 7, 7]),
            in1=r3(Pex[:]).transpose([0, 2, 1]),
            op=OP.add)
        nc.vector.tensor_reduce(out=ent[:].unsqueeze(2),
                                in_=r3(t2[:]),
                                axis=mybir.AxisListType.X, op=OP.max)

        # ---- pass 3 x rounds: fv within blocks; bps on last round ----
        bps = st.tile([P, S * T], F32, tag="bps")
        v = st.tile([P, T], F32, tag="v")
        for r in range(rounds):
            nc.vector.tensor_copy(out=v[:], in_=ent[:])
            for s in range(S):
                sc = pp.tile([P, 49], F32, tag="sc")
                nc.vector.tensor_tensor(
                    out=r3(sc[:]),
                    in0=v[:].unsqueeze(1).broadcast_to([P, 7, 7]),
                    in1=trans_v, op=OP.add)
                m1 = pp.tile([P, T], F32, tag="m7")
                nc.vector.tensor_reduce(out=m1[:].unsqueeze(2),
                                        in_=r3(sc[:]),
                                        axis=mybir.AxisListType.X, op=OP.max)
                if r == rounds - 1:
                    mk = pp.tile([P, 49], F32, tag="mk")
                    nc.vector.tensor_tensor(
                        out=r3(mk[:]), in0=r3(sc[:]),
                        in1=m1[:].unsqueeze(2).broadcast_to([P, 7, 7]),
                        op=OP.is_lt)
                    nc.vector.tensor_scalar_mul(mk[:], mk[:], 1000.0)
                    nc.vector.tensor_tensor(out=mk[:], in0=mk[:],
                                            in1=cons["iota_p49"][:], op=OP.add)
                    nc.vector.tensor_reduce(
                        out=bps[:, s * T:(s + 1) * T].unsqueeze(2),
                        in_=r3(mk[:]), axis=mybir.AxisListType.X, op=OP.min)
                nc.vector.tensor_tensor(out=v[:], in0=m1[:],
                                        in1=fsb[:, s * T:(s + 1) * T], op=OP.add)
            if r < rounds - 1:
                ent2 = st.tile([P, T], F32, tag=f"ent{r}")
                nc.sync.dma_start(ent2[1:P, :], v[0:P - 1, :])
                nc.sync.dma_start(ent2[0:1, :], cons["init7"][0:1, :])
                ent = ent2

        # ---- terminal: best tag (first-index argmax) ----
        tl = st.tile([P, T], F32, tag="tl")
        nc.vector.tensor_tensor(out=tl[:], in0=v[:], in1=cons["trSTOP"][:], op=OP.add)
        sc0 = st.tile([P, 1], F32, tag="sc0")
        nc.vector.tensor_reduce(out=sc0[:], in_=tl[:], axis=mybir.AxisListType.X,
                                op=OP.max)
        mkT = pp.tile([P, T], F32, tag="mkT")
        nc.vector.tensor_tensor(out=mkT[:], in0=tl[:],
                                in1=sc0[:].broadcast_to([P, T]), op=OP.is_lt)
        nc.vector.tensor_scalar_mul(mkT[:], mkT[:], 1000.0)
        nc.vector.tensor_tensor(out=mkT[:], in0=mkT[:], in1=cons["iota7"][:], op=OP.add)
        bestf = st.tile([P, 1], F32, tag="bestf")
        nc.vector.tensor_reduce(out=bestf[:], in_=mkT[:], axis=mybir.AxisListType.X,
                                op=OP.min)
        # broadcast best (partition 127) to all partitions via K=1 matmul
        b0 = st.tile([P, 1], F32, tag="b0")
        nc.sync.dma_start(b0[0:1, :], bestf[127:128, :])
        ones_r = st.tile([1, P], F32, tag="ones_r")
        nc.gpsimd.memset(ones_r[:], 1.0)
        bb_ps = ps_p.tile([P, 1], F32, space="PSUM")
        nc.tensor.matmul(out=bb_ps[:], lhsT=ones_r[:], rhs=b0[0:1, :],
                         start=True, stop=True)
        best_bc = st.tile([P, 1], F32, tag="best_bc")
        nc.vector.tensor_copy(out=best_bc[:], in_=bb_ps[:])

        def sel7(table, index, tag):
            # out[p,1] = table[p, index[p]] ; table [P,7], index [P,1] float
            eq = pp.tile([P, T], F32, tag="eqs")
            nc.vector.tensor_tensor(out=eq[:], in0=cons["iota7"][:],
                                    in1=index.broadcast_to([P, T]), op=OP.is_equal)
            nc.vector.tensor_tensor(out=eq[:], in0=eq[:], in1=table, op=OP.mult)
            out = st.tile([P, 1], F32, tag=tag)
            nc.vector.tensor_reduce(out=out[:], in_=eq[:],
                                    axis=mybir.AxisListType.X, op=OP.add)
            return out

        # ---- compose per-block backtrack maps G_b = bp_0 o ... o bp_31 ----
        M = st.tile([P, T], F32, tag="Mmap")
        nc.vector.tensor_copy(out=M[:], in_=cons["iota7"][:])
        for s in range(S - 1, -1, -1):
            eq = pp.tile([P, 49], F32, tag="eqm")
            nc.vector.tensor_tensor(
                out=r3(eq[:]),
                in0=M[:].unsqueeze(2).broadcast_to([P, 7, 7]),
                in1=iota_p_v, op=OP.is_equal)
            nc.vector.tensor_tensor(
                out=r3(eq[:]), in0=r3(eq[:]),
                in1=bps[:, s * T:(s + 1) * T].unsqueeze(1).broadcast_to([P, 7, 7]),
                op=OP.mult)
            Mn = pp.tile([P, T], F32, tag="Mn")
            nc.vector.tensor_reduce(out=Mn[:].unsqueeze(2),
                                    in_=r3(eq[:]), axis=mybir.AxisListType.X,
                                    op=OP.add)
            nc.vector.tensor_copy(out=M[:], in_=Mn[:])

        # suffix-compose: S_b = Q_b o Q_{b+1} o ... ; Q_b = G_{b+1}, Q_127 = id
        Sm = st.tile([P, T], F32, tag="Sm")
        nc.sync.dma_start(Sm[0:P - 1, :], M[1:P, :])
        nc.sync.dma_start(Sm[P - 1:P, :], cons["iota7"][0:1, :])
        for d in (1, 2, 4, 8, 16, 32, 64):
            Ssh = pp.tile([P, T], F32, tag="Ssh")
            nc.sync.dma_start(Ssh[0:P - d, :], Sm[d:P, :])
            nc.sync.dma_start(Ssh[P - d:P, :], cons["iota7"][0:d, :])
            eq = pp.tile([P, 49], F32, tag="eqs2")
            nc.vector.tensor_tensor(
                out=r3(eq[:]),
                in0=Ssh[:].unsqueeze(2).broadcast_to([P, 7, 7]),
                in1=iota_p_v, op=OP.is_equal)
            nc.vector.tensor_tensor(
                out=r3(eq[:]), in0=r3(eq[:]),
                in1=Sm[:].unsqueeze(1).broadcast_to([P, 7, 7]), op=OP.mult)
            Sn = pp.tile([P, T], F32, tag="Sn")
            nc.vector.tensor_reduce(out=Sn[:].unsqueeze(2),
                                    in_=r3(eq[:]), axis=mybir.AxisListType.X,
                                    op=OP.add)
            nc.vector.tensor_copy(out=Sm[:], in_=Sn[:])
        tag_end = sel7(Sm[:], best_bc[:], "tag_end")

        # ---- final within-block backtrack + path + score terms ----
        paths = st.tile([P, S], F32, tag="paths")
        acc = st.tile([P, 1], F32, tag="acc")
        nc.gpsimd.memset(acc[:], 0.0)
        tag = tag_end
        for s in range(S - 1, -1, -1):
            nc.vector.tensor_copy(out=paths[:, s:s + 1], in_=tag[:])
            tag_prev = sel7(bps[:, s * T:(s + 1) * T], tag[:], f"tp{s % 2}")
            i1 = pp.tile([P, 1], F32, tag="i1")
            nc.vector.tensor_scalar_mul(i1[:], tag[:], 7.0)
            nc.vector.tensor_tensor(out=i1[:], in0=i1[:], in1=tag_prev[:], op=OP.add)
            eq49 = pp.tile([P, 49], F32, tag="eq49")
            nc.vector.tensor_tensor(out=eq49[:], in0=cons["iota49l"][:],
                                    in1=i1[:].broadcast_to([P, 49]), op=OP.is_equal)
            nc.vector.tensor_tensor(out=eq49[:], in0=eq49[:], in1=cons["trans49"][:],
                                    op=OP.mult)
            tval = pp.tile([P, 1], F32, tag="tval")
            nc.vector.tensor_reduce(out=tval[:], in_=eq49[:],
                                    axis=mybir.AxisListType.X, op=OP.add)
            fval = sel7(fsb[:, s * T:(s + 1) * T], tag[:], "fval")
            nc.vector.tensor_tensor(out=acc[:], in0=acc[:], in1=tval[:], op=OP.add)
            nc.vector.tensor_tensor(out=acc[:], in0=acc[:], in1=fval[:], op=OP.add)
            tag = tag_prev
        # score = sum_partitions(acc) + trans[STOP, best]
        ones_c = st.tile([P, 1], F32, tag="ones_c")
        nc.gpsimd.memset(ones_c[:], 1.0)
        sc_ps = ps_p.tile([1, 1], F32, space="PSUM")
        nc.tensor.matmul(out=sc_ps[:], lhsT=ones_c[:], rhs=acc[:], start=True,
                         stop=True)
        stv = sel7(cons["trSTOP"][:], best_bc[:], "stv")
        tot = st.tile([1, 1], F32, tag="tot")
        nc.vector.tensor_copy(out=tot[:], in_=sc_ps[:])
        nc.vector.tensor_tensor(out=tot[:], in0=tot[:], in1=stv[0:1, :], op=OP.add)
        nc.sync.dma_start(score_out[:], tot[:])

        pi = st.tile([P, S], I32, tag="pi")
        nc.vector.tensor_copy(out=pi[:], in_=paths[:])
        nc.sync.dma_start(path_out[:], pi[:])

    nc.compile()
    return nc


# --------------------------------------------------------------------------
# Host orchestration
# --------------------------------------------------------------------------
def _bf(x):
    return x.astype(ml_dtypes.bfloat16)


def _hilo(x):
    h = _bf(x)
    l = _bf((x - h.astype(np.float32)).astype(np.float32))
    return h, l


_PROGS = {}


def _get_progs():
    if "lstm" not in _PROGS:
        _PROGS["lstm"] = build_lstm()
        _PROGS["vit"] = build_viterbi()
    return _PROGS["lstm"], _PROGS["vit"]


def _run_spmd(nc, in_maps, n_cores, bench_reps=0):
    """Execute a compiled Bass program on the first n_cores devices via PJRT
    (mirrors bass2jax.run_bass_via_pjrt) with device-resident inputs. If
    bench_reps > 0, re-executes and returns an amortized per-run wall time."""
    import time
    import jax
    from jax.sharding import Mesh, PartitionSpec, NamedSharding
    from concourse import bass2jax, mybir as mb
    bass2jax.install_neuronx_cc_hook()

    part_name = nc.partition_id_tensor.name if nc.partition_id_tensor else None
    in_names, out_names, out_avals, zero_outs = [], [], [], []
    for alloc in nc.m.functions[0].allocations:
        if not isinstance(alloc, mb.MemoryLocationSet):
            continue
        name = alloc.memorylocations[0].name
        if alloc.kind == "ExternalInput":
            if name != part_name:
                in_names.append(name)
        elif alloc.kind == "ExternalOutput":
            shape = tuple(alloc.tensor_shape)
            dtype = mb.dt.np(alloc.dtype)
            out_names.append(name)
            out_avals.append(jax.core.ShapedArray(shape, dtype))
            zero_outs.append(np.zeros(shape, dtype))
    n_params, n_outs = len(in_names), len(out_avals)
    all_in = in_names + out_names
    if part_name is not None:
        all_in = all_in + [part_name]
    donate = tuple(range(n_params, n_params + n_outs))

    def _body(*args):
        operands = list(args)
        if part_name is not None:
            operands.append(bass2jax.partition_id_tensor())
        return tuple(bass2jax._bass_exec_p.bind(
            *operands, out_avals=tuple(out_avals), in_names=tuple(all_in),
            out_names=tuple(out_names), lowering_input_output_aliases=(),
            sim_require_finite=True, sim_require_nnan=True, nc=nc))

    devices = jax.devices()[:n_cores]
    if n_cores == 1:
        dev = devices[0]
        fn = jax.jit(_body, donate_argnums=donate, keep_unused=True)
        dev_in = [jax.device_put(np.asarray(in_maps[0][nm]), dev) for nm in in_names]

        def run_once():
            zs = [jax.device_put(z, dev) for z in zero_outs]
            return fn(*dev_in, *zs)
    else:
        from jax.experimental.shard_map import shard_map
        mesh = Mesh(np.asarray(devices), ("core",))
        spec = NamedSharding(mesh, PartitionSpec("core"))
        in_specs = (PartitionSpec("core"),) * (n_params + n_outs)
        out_specs = (PartitionSpec("core"),) * n_outs
        fn = jax.jit(shard_map(_body, mesh=mesh, in_specs=in_specs,
                               out_specs=out_specs, check_rep=False),
                     donate_argnums=donate, keep_unused=True)
        concat_in = [np.concatenate([np.asarray(in_maps[c][nm])
                                     for c in range(n_cores)], axis=0)
                     for nm in in_names]
        dev_in = [jax.device_put(a, spec) for a in concat_in]
        del concat_in
        czeros = [np.zeros((n_cores * z.shape[0], *z.shape[1:]), z.dtype)
                  for z in zero_outs]

        def run_once():
            zs = [jax.device_put(z, spec) for z in czeros]
            return fn(*dev_in, *zs)

    outs = jax.block_until_ready(run_once())
    per_run_ns = None
    if bench_reps > 0:
        jax.block_until_ready(run_once())
        t0 = time.perf_counter()
        last = [run_once() for _ in range(bench_reps)]
        jax.block_until_ready(last)
        t1 = time.perf_counter()
        per_run_ns = (t1 - t0) / bench_reps * 1e9
    out_np = [np.asarray(o) for o in outs]
    if n_cores == 1:
        results = [{nm: out_np[i] for i, nm in enumerate(out_names)}]
    else:
        results = [{nm: out_np[i].reshape(n_cores, *out_avals[i].shape)[c]
                    for i, nm in enumerate(out_names)} for c in range(n_cores)]
    return results, per_run_ns


def kernel(sentence, emb, w_ih_f, w_hh_f, b_ih_f, b_hh_f,
           w_ih_b, w_hh_b, b_ih_b, b_hh_b, w_out, b_out, transitions):
    global LAST_RESULTS
    LAST_RESULTS = []
    sent_np = np.asarray(sentence)
    out_int_dtype = sent_np.dtype if sent_np.dtype in (np.dtype(np.int32),
                                                       np.dtype(np.int64)) else np.int64
    sent = sent_np.astype(np.int64)
    emb = np.asarray(emb, np.float32)
    w_out = np.asarray(w_out, np.float32)
    b_out = np.asarray(b_out, np.float32)
    transitions = np.asarray(transitions, np.float32)

    nc1, nc2 = _get_progs()

    def dir_inputs(seq, w_ih, w_hh, b_ih, b_hh, wo_half):
        wih_hi, wih_lo = _hilo(np.asarray(w_ih, np.float32).T.copy())
        whh_hi, whh_lo = _hilo(np.asarray(w_hh, np.float32).T.copy())
        wo_hi, wo_lo = _hilo(np.ascontiguousarray(wo_half.T))
        bias = (np.asarray(b_ih, np.float32) + np.asarray(b_hh, np.float32))
        bias_t = bias.reshape(MT, P).T.copy()
        maps = []
        for n in range(L // CHUNK):
            lo = n * CHUNK - HALO
            if lo < 0:
                idx = np.concatenate([np.zeros(-lo, np.int64), seq[0:(n + 1) * CHUNK]])
                mask = np.zeros((P, HALO), np.float32)
            else:
                idx = seq[lo:(n + 1) * CHUNK]
                mask = np.ones((P, HALO), np.float32)
            maps.append({
                "emb": emb, "idx": idx.astype(np.int32).reshape(LC, 1),
                "mask128": mask,
                "wih_hi": wih_hi, "wih_lo": wih_lo,
                "whh_hi": whh_hi, "whh_lo": whh_lo,
                "bias": bias_t, "wo_hi": wo_hi, "wo_lo": wo_lo,
            })
        return maps

    in_maps = (dir_inputs(sent, w_ih_f, w_hh_f, b_ih_f, b_hh_f, w_out[:, :H])
               + dir_inputs(sent[::-1], w_ih_b, w_hh_b, b_ih_b, b_hh_b, w_out[:, H:]))

    import os as _os
    reps = int(_os.environ.get("KERNEL_BENCH", "0"))
    res1, t1ns = _run_spmd(nc1, in_maps, NCORES, bench_reps=reps)
    LAST_RESULTS.append(t1ns)
    featsF = np.concatenate([res1[c]["feats_out"] for c in range(4)], axis=0)
    featsBr = np.concatenate([res1[c]["feats_out"] for c in range(4, 8)], axis=0)
    featsB = featsBr[::-1].copy()

    # launch 2 constants
    ii = np.arange(49, dtype=np.float32)
    id49 = np.where((ii // 7) == (ii % 7), 0.0, NEG2).astype(np.float32)
    init7 = np.full(T, NEG, np.float32); init7[START] = 0.0
    in2 = {
        "fA": featsF.astype(np.float32),
        "fB": featsB.astype(np.float32),
        "b_out_t": np.tile(b_out, (P, S)).astype(np.float32),
        "trans49": np.tile(transitions.reshape(1, 49), (P, 1)).astype(np.float32),
        "iota_p49": np.tile((ii % 7), (P, 1)).astype(np.float32),
        "iota49l": np.tile(ii, (P, 1)).astype(np.float32),
        "id49": np.tile(id49, (P, 1)).astype(np.float32),
        "iota7": np.tile(np.arange(T, dtype=np.float32), (P, 1)),
        "init7": np.tile(init7, (P, 1)),
        "trSTOP": np.tile(transitions[STOP], (P, 1)).astype(np.float32),
    }
    res2, t2ns = _run_spmd(nc2, [in2], 1, bench_reps=reps)
    LAST_RESULTS.append(t2ns)
    path = res2[0]["path_out"].reshape(L).astype(out_int_dtype)
    score = np.float32(res2[0]["score_out"][0, 0])
    return path, score
